# revision 11
# baseline (speedup 1.0000x reference)
"""Trainium2 Bass kernel for nn_Encoder (GNN message passing / MeshGraphNet-style encoder).

Strategy (8 NeuronCores, SPMD one NEFF):
  - Per batch (B=2), nodes are split into 8 contiguous-by-edge-count core shards
    (balanced so each core gets ~Ne/8 edges). Within a core, nodes are bin-packed
    into WPC windows of 128 slots such that every window owns <= EW edges
    (receiver-sharded graph partition -> segment-sum is core-local AND
    window-local: no cross-core reduction for the scatter).
  - Each round: edge phase streams windows: indirect-DMA gathers of sender /
    receiver V rows, PE transposes to feature-major, 2-layer MLP on PE
    (L0 feature-major, L1 row-major via hidden-as-lhsT), LayerNorm via
    bn_stats + fused ACT normalize, segment-sum as one-hot matmuls into PSUM,
    E residual updated feature-major with a single fused scalar_tensor_tensor.
  - Node phase: per-window MLP on the core's own V shard (V transposed on PE),
    LayerNorm, residual. V shards are AllGather'ed into a shared V table for
    the next round's gathers. Batches are interleaved so the AllGather of one
    batch hides behind the other batch's compute.
  - LayerNorm gamma/beta are folded host-side: beta shifts go into downstream
    L0 biases (+ final host-side add), gamma of e_new into the aggregator
    weight W0na' = diag(gamma_e) @ W0na; per-node beta*deg rank-1 terms ride a
    "deg" row appended to the node pos input. The device only ever tracks
    beta-free residual streams.

kernel(**inputs) takes the FULL inputs and returns (V, E) like the reference.
"""

import math
import os

import numpy as np

H = 128
LN_EPS = 1e-5


# ---------------------------------------------------------------------------
# configuration
# ---------------------------------------------------------------------------

def make_cfg(B, N, Ne, ncores=8, wpc=None, ew=None, g=None):
    cfg = {}
    cfg["B"], cfg["N"], cfg["Ne"], cfg["NCORES"] = B, N, Ne, ncores
    # windows per core: enough 128-slots to hold N/ncores nodes plus slack
    if wpc is None:
        wpc = int(math.ceil((N / ncores) * 1.02 / 128)) + 1
    cfg["WPC"] = wpc
    cfg["NSLOT"] = wpc * 128          # node slots per core
    cfg["NTOT"] = ncores * wpc * 128  # global V-table rows
    # edges per window (multiple of 128)
    if ew is None:
        mean_ew = (Ne / ncores) / wpc
        ew = int(math.ceil(mean_ew * 1.12 / 128)) * 128
    cfg["EW"] = ew
    cfg["NCH"] = ew // 128
    cfg["ESH"] = wpc * ew             # edge slots per core
    cfg["CHT"] = wpc * (ew // 128)    # chunks total per core
    if g is None:
        g = 4
        while wpc % g:
            g -= 1
    cfg["G"] = g                      # windows per gather group
    cfg["NGRP"] = wpc // g
    cfg["NROUNDS"] = 4
    return cfg


# ---------------------------------------------------------------------------
# host-side graph partitioning / data layout
# ---------------------------------------------------------------------------

def _pack_windows(node_deg, wpc, ew):
    """FFD-pack nodes (item size = degree) into wpc bins with caps
    (ew edges, 128 slots). Returns list of lists of node positions, or None."""
    nn = len(node_deg)
    order = np.argsort(-node_deg, kind="stable")
    rem_e = np.full(wpc, ew, dtype=np.int64)
    rem_s = np.full(wpc, 128, dtype=np.int64)
    assign = np.empty(nn, dtype=np.int64)
    for i in order:
        d = node_deg[i]
        # worst-fit by remaining edge capacity among bins with a free slot
        cand = np.where((rem_s > 0) & (rem_e >= d))[0]
        if len(cand) == 0:
            return None
        w = cand[np.argmax(rem_e[cand])]
        assign[i] = w
        rem_e[w] -= d
        rem_s[w] -= 1
    return assign


def prep_batch(b, edges, mesh_pos, states, node_type, pos_enc, cfg):
    """Builds all per-core device arrays + unshard metadata for batch b."""
    N, Ne, NCORES, WPC = cfg["N"], cfg["Ne"], cfg["NCORES"], cfg["WPC"]
    EW, NCH, ESH, CHT, NSLOT = cfg["EW"], cfg["NCH"], cfg["ESH"], cfg["CHT"], cfg["NSLOT"]

    s = np.asarray(edges[b, :, 0], dtype=np.int64)
    r = np.asarray(edges[b, :, 1], dtype=np.int64)
    deg = np.bincount(r, minlength=N).astype(np.int64)

    # --- core split: contiguous node ranges balanced by edge count ---
    cum = np.cumsum(deg)
    bounds = [0]
    for c in range(1, NCORES):
        t = Ne * c / NCORES
        bounds.append(int(np.searchsorted(cum, t)))
    bounds.append(N)
    core_of = np.empty(N, dtype=np.int64)
    for c in range(NCORES):
        core_of[bounds[c]:bounds[c + 1]] = c
        assert bounds[c + 1] - bounds[c] <= NSLOT, "core node overflow"

    # --- window packing per core ---
    pi = np.full(N, -1, dtype=np.int64)          # node -> global slot id
    slot_nodes = np.full((NCORES, NSLOT), -1, dtype=np.int64)  # slot -> node
    for c in range(NCORES):
        nodes = np.arange(bounds[c], bounds[c + 1])
        assign = _pack_windows(deg[nodes], WPC, EW)
        assert assign is not None, "window packing failed; raise EW"
        # slot within window = order of assignment
        for w in range(WPC):
            members = nodes[assign == w]
            k = len(members)
            slot_nodes[c, w * 128:w * 128 + k] = members
            pi[members] = c * NSLOT + w * 128 + np.arange(k)

    # --- edge placement ---
    ecore = core_of[r]
    ewnd = (pi[r] % NSLOT) // 128                # window within core
    key = ecore * WPC + ewnd
    eorder = np.argsort(key, kind="stable")
    key_s = key[eorder]
    # index within bucket
    uniq, start_idx = np.unique(key_s, return_index=True)
    within = np.arange(Ne) - np.repeat(start_idx, np.diff(np.append(start_idx, Ne)))
    esl = np.empty(Ne, dtype=np.int64)           # edge -> slot within its core
    esl[eorder] = (key_s % WPC) * EW + within
    assert within.max() < EW, "window edge overflow"

    per_core = []
    for c in range(NCORES):
        mask = ecore == c
        eidx = np.where(mask)[0]
        sl = esl[eidx]
        d = {}
        soff = np.zeros(ESH, dtype=np.int32)
        soff[sl] = pi[s[eidx]]
        # [128, CHT] layout: (p, ch) -> slot ch*128+p
        d["soff"] = soff.reshape(CHT, 128).T.copy()

        oh = np.zeros((ESH, 128), dtype=np.float32)
        oh[sl, pi[r[eidx]] % 128] = 1.0
        # [128, CHT*128]: partition p = slot%128, cols ch*128 + target
        d["oh"] = (
            oh.reshape(CHT, 128, 128).transpose(1, 0, 2).reshape(128, CHT * 128).copy()
        )
        # transposed one-hot [128 node-slot, WPC*EW]: ohT[n, w*EW+e] = oh[w*EW+e, n]
        d["ohT"] = oh.reshape(WPC, EW, 128).transpose(2, 0, 1).reshape(128, WPC * EW).copy()

        poss = np.zeros((57, ESH), dtype=np.float32)
        poss[:56, sl] = pos_enc[b, s[eidx]].T
        poss[56, :] = 1.0
        d["poss"] = poss
        posr = np.zeros((56, ESH), dtype=np.float32)
        posr[:, sl] = pos_enc[b, r[eidx]].T
        d["posr"] = posr

        geom = np.zeros((4, ESH), dtype=np.float32)
        dist = mesh_pos[b, s[eidx]] - mesh_pos[b, r[eidx]]
        nrm = np.sqrt(np.sum(dist * dist, axis=-1))
        geom[0:2, sl] = dist.T
        geom[2, sl] = nrm
        geom[3, :] = 1.0
        d["geom"] = geom

        nodes = slot_nodes[c]
        valid = nodes >= 0
        nv = nodes[valid]
        nodef = np.zeros((13, NSLOT), dtype=np.float32)
        nodef[0:3, valid] = states[b, nv].T
        nodef[3:12, valid] = node_type[b, nv].T
        nodef[12, :] = 1.0
        d["nodef"] = nodef
        posn = np.zeros((58, NSLOT), dtype=np.float32)
        posn[:56, valid] = pos_enc[b, nv].T
        posn[56, :] = 1.0
        posn[57, valid] = deg[nv].astype(np.float32)
        d["posn"] = posn

        # unshard metadata
        d["edge_ids"] = eidx       # original edge ids (this core)
        d["edge_slots"] = sl       # their slots
        d["node_ids"] = nv
        d["node_slots"] = np.where(valid)[0]
        per_core.append(d)
    return per_core


def fold_weights(params):
    """Host-side gamma/beta folding. Returns dict of device weight arrays."""
    f32 = lambda a: np.asarray(a, dtype=np.float32)
    W = {}
    en = params["enc_node"]
    W["wgn"] = np.vstack([f32(en["w0"]), f32(en["b0"])[None]])       # [13,128]
    W["w1gn"] = f32(en["w1"])
    W["b1gn"] = f32(en["b1"])[None]                                   # [1,128]
    ee = params["enc_edge"]
    W["wge"] = np.vstack([f32(ee["w0"]), f32(ee["b0"])[None]])        # [4,128]
    W["w1ge"] = f32(ee["w1"])
    W["b1ge"] = f32(ee["b1"])[None]

    Bv = np.zeros(H, np.float32)
    Be = np.zeros(H, np.float32)
    for k, gp in enumerate(params["gn"]):
        fe, fn = gp["f_edge"], gp["f_node"]
        w0e = f32(fe["w0"])
        w0vs, w0ps = w0e[0:128], w0e[128:184]
        w0vr, w0pr = w0e[184:312], w0e[312:368]
        w0ee = w0e[368:496]
        b0fold = f32(fe["b0"]) + Bv @ (w0vs + w0vr) + Be @ w0ee
        W[f"wvs{k}"] = w0vs
        W[f"wvr{k}"] = w0vr
        W[f"wps{k}"] = np.vstack([w0ps, b0fold[None]])                # [57,128]
        W[f"wpr{k}"] = w0pr
        W[f"wee{k}"] = w0ee
        W[f"w1e{k}"] = f32(fe["w1"])
        W[f"b1e{k}"] = f32(fe["b1"])[None]
        game, betae = f32(fe["ln_s"]), f32(fe["ln_b"])
        W[f"game{k}"] = game[:, None]                                 # [128,1]

        w0n = f32(fn["w0"])
        w0nv, w0np, w0na = w0n[0:128], w0n[128:184], w0n[184:312]
        b0nfold = f32(fn["b0"]) + Bv @ w0nv
        c_k = betae @ w0na
        W[f"wnv{k}"] = w0nv
        W[f"wnp{k}"] = np.vstack([w0np, b0nfold[None], c_k[None]])    # [58,128]
        W[f"wna{k}"] = game[:, None] * w0na                           # [128,128]
        W[f"w1n{k}"] = f32(fn["w1"])
        W[f"b1n{k}"] = f32(fn["b1"])[None]
        gamv, betav = f32(fn["ln_s"]), f32(fn["ln_b"])
        W[f"gamv{k}"] = np.tile(gamv[None, :], (128, 1))              # [128,128]
        Bv = Bv + betav
        Be = Be + betae
    W["_Bv"], W["_Be"] = Bv, Be
    return W


# ---------------------------------------------------------------------------
# device kernel trace
# ---------------------------------------------------------------------------

def build_trace(tc, ins, outs, cfg):
    """Emit the full kernel under an entered TileContext `tc`.
    ins/outs: dicts name -> bass.AP (DRAM)."""
    import concourse.bass as bass
    import concourse.tile as tile  # noqa: F401
    from concourse import mybir
    from concourse.masks import make_identity

    nc = tc.nc
    f32 = mybir.dt.float32
    AF = mybir.ActivationFunctionType
    OP = mybir.AluOpType
    B, WPC, EW, NCH, ESH, CHT = (
        cfg["B"], cfg["WPC"], cfg["EW"], cfg["NCH"], cfg["ESH"], cfg["CHT"],
    )
    NSLOT, NTOT, G, NGRP, NR = (
        cfg["NSLOT"], cfg["NTOT"], cfg["G"], cfg["NGRP"], cfg["NROUNDS"],
    )
    NCORES = cfg["NCORES"]
    RG = [list(range(NCORES))]

    # internal DRAM state
    vtab = [nc.dram_tensor(f"vtab{b}", [NTOT, H], f32, kind="Internal",
                           addr_space="Shared") for b in range(B)]
    vsh = [nc.dram_tensor(f"vsh{b}", [NSLOT, H], f32, kind="Internal")
           for b in range(B)]

    consts = tc.alloc_tile_pool(name="consts", bufs=1)
    sb = tc.alloc_tile_pool(name="sb", bufs=3)
    gath = tc.alloc_tile_pool(name="gath", bufs=2)
    aggp = tc.alloc_tile_pool(name="aggp", bufs=2)
    psA = tc.alloc_tile_pool(name="psA", bufs=6, space="PSUM")
    psB = tc.alloc_tile_pool(name="psB", bufs=2, space="PSUM")

    # ---- constants ----
    ident = consts.tile([128, 128], f32)
    make_identity(nc, ident[:])
    ones_row = consts.tile([1, 128], f32)
    nc.vector.memset(ones_row[:], 1.0)
    ones_ew = consts.tile([1, EW], f32)
    nc.vector.memset(ones_ew[:], 1.0)
    epsc = consts.tile([128, 1], f32)
    nc.vector.memset(epsc[:], LN_EPS)

    def cload(name, shape, dtype=f32):
        t = consts.tile(shape, dtype, tag=f"c_{name}")
        nc.sync.dma_start(out=t[:], in_=ins[name][:])
        return t

    wgn = cload("wgn", [13, H]); w1gn = cload("w1gn", [H, H]); b1gn = cload("b1gn", [1, H])
    wge = cload("wge", [4, H]); w1ge = cload("w1ge", [H, H]); b1ge = cload("b1ge", [1, H])
    wvs, wvr, wps, wpr, wee, w1e, b1e, game = [], [], [], [], [], [], [], []
    wnv, wnp, wna, w1n, b1n, gamv = [], [], [], [], [], []
    for k in range(NR):
        wvs.append(cload(f"wvs{k}", [H, H])); wvr.append(cload(f"wvr{k}", [H, H]))
        wps.append(cload(f"wps{k}", [57, H])); wpr.append(cload(f"wpr{k}", [56, H]))
        wee.append(cload(f"wee{k}", [H, H])); w1e.append(cload(f"w1e{k}", [H, H]))
        b1e.append(cload(f"b1e{k}", [1, H])); game.append(cload(f"game{k}", [H, 1]))
        wnv.append(cload(f"wnv{k}", [H, H])); wnp.append(cload(f"wnp{k}", [58, H]))
        wna.append(cload(f"wna{k}", [H, H])); w1n.append(cload(f"w1n{k}", [H, H]))
        b1n.append(cload(f"b1n{k}", [1, H])); gamv.append(cload(f"gamv{k}", [H, H]))
    soff = [cload(f"soff{b}", [128, CHT], mybir.dt.int32) for b in range(B)]

    def layernorm_cols(chunks):
        """Stats for row-major PSUM y chunks (list of [128, H] APs)
        -> (rstd, nmr) [128, nchunks]."""
        nchunks = len(chunks)
        stats = sb.tile([128, nchunks, 6], f32, tag="ln_stats")
        mv = sb.tile([128, nchunks, 2], f32, tag="ln_mv")
        for c in range(nchunks):
            nc.vector.bn_stats(stats[:, c, :], chunks[c])
            nc.vector.bn_aggr(mv[:, c, :], stats[:, c, :])
        std = sb.tile([128, nchunks], f32, tag="ln_std")
        nc.scalar.activation(std[:], mv[:, :, 1], AF.Sqrt, bias=epsc[:])
        rstd = sb.tile([128, nchunks], f32, tag="ln_rstd")
        nc.vector.reciprocal(rstd[:], std[:])
        nmr = sb.tile([128, nchunks], f32, tag="ln_nmr")
        nc.vector.scalar_tensor_tensor(
            nmr[:], in0=mv[:, :, 0], scalar=-1.0, in1=rstd[:],
            op0=OP.mult, op1=OP.mult)
        return rstd, nmr

    # ---- encoders ----
    for b in range(B):
        # edge encoder -> E0 (feature-major) into Eout
        for w in range(WPC):
            gsl = slice(w * EW, (w + 1) * EW)
            geom = sb.tile([4, EW], f32, tag="geom")
            nc.sync.dma_start(out=geom[:], in_=ins[f"geom{b}"][:, gsl])
            ph = psA.tile([H, EW], f32, tag="psA")
            nc.tensor.matmul(ph[:], lhsT=wge[:], rhs=geom[:], start=True, stop=True)
            hid = sb.tile([H, EW], f32, tag="hid")
            nc.scalar.activation(hid[:], ph[:], AF.Relu)
            pe0 = psA.tile([H, EW], f32, tag="psA")
            nc.tensor.matmul(pe0[:], lhsT=w1ge[:], rhs=hid[:], start=True, stop=False)
            nc.tensor.matmul(pe0[:], lhsT=b1ge[:], rhs=ones_ew[:], start=False, stop=True)
            e0 = sb.tile([H, EW], f32, tag="enew")
            nc.vector.tensor_copy(e0[:], pe0[:])
            nc.sync.dma_start(out=outs[f"Eout{b}"][:, gsl], in_=e0[:])
        # node encoder -> V0 rows into vsh
        for w in range(WPC):
            nsl = slice(w * 128, (w + 1) * 128)
            nf = sb.tile([13, 128], f32, tag="nodef")
            nc.sync.dma_start(out=nf[:], in_=ins[f"nodef{b}"][:, nsl])
            ph = psB.tile([H, 128], f32, tag="psB")
            nc.tensor.matmul(ph[:], lhsT=wgn[:], rhs=nf[:], start=True, stop=True)
            hid = sb.tile([H, 128], f32, tag="hidn")
            nc.scalar.activation(hid[:], ph[:], AF.Relu)
            pv0 = psB.tile([128, H], f32, tag="psB")
            nc.tensor.matmul(pv0[:], lhsT=hid[:], rhs=w1gn[:], start=True, stop=False)
            nc.tensor.matmul(pv0[:], lhsT=ones_row[:], rhs=b1gn[:], start=False, stop=True)
            v0 = sb.tile([128, H], f32, tag="vnew")
            nc.vector.tensor_copy(v0[:], pv0[:])
            nc.sync.dma_start(out=vsh[b][nsl, :], in_=v0[:])
        nc.gpsimd.collective_compute(
            "AllGather", OP.bypass, replica_groups=RG,
            ins=[vsh[b].ap()], outs=[vtab[b].ap()])

    # ---- rounds ----
    for k in range(NR):
        for b in range(B):
            agg = aggp.tile([H, WPC * 128], f32, tag="agg")
            # ---------------- edge phase ----------------
            if True:
                for w in range(WPC):
                    gsl = slice(w * EW, (w + 1) * EW)
                    # sender V rows: per-chunk indirect gathers ([128,1] offsets)
                    gs = gath.tile([128, NCH, H], f32, tag="gs")
                    for c in range(NCH):
                        ch = w * NCH + c
                        nc.gpsimd.indirect_dma_start(
                            out=gs[:, c, :], out_offset=None, in_=vtab[b].ap(),
                            in_offset=bass.IndirectOffsetOnAxis(
                                ap=soff[b][:, ch:ch + 1], axis=0))
                    eold = sb.tile([H, EW], f32, tag="eold")
                    nc.sync.dma_start(out=eold[:], in_=outs[f"Eout{b}"][:, gsl])
                    poss = sb.tile([57, EW], f32, tag="poss")
                    nc.sync.dma_start(out=poss[:], in_=ins[f"poss{b}"][:, gsl])
                    posr = sb.tile([56, EW], f32, tag="posr")
                    nc.sync.dma_start(out=posr[:], in_=ins[f"posr{b}"][:, gsl])
                    oh = sb.tile([128, NCH * 128], f32, tag="oh")
                    nc.sync.dma_start(
                        out=oh[:],
                        in_=ins[f"oh{b}"][:, w * NCH * 128:(w + 1) * NCH * 128])
                    ohT = sb.tile([128, EW], f32, tag="ohT")
                    nc.sync.dma_start(out=ohT[:], in_=ins[f"ohT{b}"][:, gsl])
                    # receiver V rows: window-local one-hot expand (feature-major)
                    vwin = sb.tile([128, H], f32, tag="vwin")
                    nc.sync.dma_start(
                        out=vwin[:], in_=vsh[b][w * 128:(w + 1) * 128, :])
                    pvr = psA.tile([H, EW], f32, tag="psA")
                    nc.tensor.matmul(pvr[:], lhsT=vwin[:], rhs=ohT[:],
                                     start=True, stop=True)
                    vrT = sb.tile([H, EW], f32, tag="vrT")
                    nc.vector.tensor_copy(vrT[:], pvr[:])
                    # sender rows -> feature-major via PE transpose
                    pvs = psA.tile([H, EW], f32, tag="psA")
                    for c in range(NCH):
                        nc.tensor.transpose(
                            pvs[:, c * 128:(c + 1) * 128], gs[:, c, :], ident[:])
                    vsT = sb.tile([H, EW], f32, tag="vsT")
                    nc.scalar.activation(vsT[:], pvs[:], AF.Copy)
                    # L0 (feature-major out)
                    ph = psA.tile([H, EW], f32, tag="psA")
                    nc.tensor.matmul(ph[:], lhsT=wvs[k][:], rhs=vsT[:], start=True, stop=False)
                    nc.tensor.matmul(ph[:], lhsT=wvr[k][:], rhs=vrT[:], start=False, stop=False)
                    nc.tensor.matmul(ph[:], lhsT=wps[k][:], rhs=poss[:], start=False, stop=False)
                    nc.tensor.matmul(ph[:], lhsT=wpr[k][:], rhs=posr[:], start=False, stop=False)
                    nc.tensor.matmul(ph[:], lhsT=wee[k][:], rhs=eold[:], start=False, stop=True)
                    hid = sb.tile([H, EW], f32, tag="hid")
                    nc.scalar.activation(hid[:], ph[:], AF.Relu)
                    # L1 (row-major out) + b1
                    py = psA.tile([128, NCH, H], f32, tag="psA")
                    for c in range(NCH):
                        nc.tensor.matmul(py[:, c, :], lhsT=hid[:, c * 128:(c + 1) * 128],
                                         rhs=w1e[k][:], start=True, stop=False)
                        nc.tensor.matmul(py[:, c, :], lhsT=ones_row[:], rhs=b1e[k][:],
                                         start=False, stop=True)
                    rstd, nmr = layernorm_cols([py[:, c, :] for c in range(NCH)])
                    normed = sb.tile([128, NCH, H], f32, tag="normed")
                    for c in range(NCH):
                        nc.scalar.activation(normed[:, c, :], py[:, c, :], AF.Identity,
                                             bias=nmr[:, c:c + 1], scale=rstd[:, c:c + 1])
                    # scatter (segment sum) into this window's agg columns
                    pagg = psB.tile([H, 128], f32, tag="psB")
                    for c in range(NCH):
                        nc.tensor.matmul(pagg[:], lhsT=normed[:, c, :],
                                         rhs=oh[:, c * 128:(c + 1) * 128],
                                         start=(c == 0), stop=(c == NCH - 1))
                    nc.scalar.activation(agg[:, w * 128:(w + 1) * 128], pagg[:], AF.Copy)
                    # E residual (feature-major, gamma as per-partition scalar)
                    pnt = psA.tile([H, EW], f32, tag="psA")
                    for c in range(NCH):
                        nc.tensor.transpose(
                            pnt[:, c * 128:(c + 1) * 128], normed[:, c, :], ident[:])
                    enew = sb.tile([H, EW], f32, tag="enew")
                    nc.vector.scalar_tensor_tensor(
                        enew[:], in0=pnt[:], scalar=game[k][:], in1=eold[:],
                        op0=OP.mult, op1=OP.add)
                    nc.sync.dma_start(out=outs[f"Eout{b}"][:, gsl], in_=enew[:])
            # ---------------- node phase ----------------
            for w in range(WPC):
                nsl = slice(w * 128, (w + 1) * 128)
                vold = sb.tile([128, H], f32, tag="vold")
                nc.sync.dma_start(out=vold[:], in_=vsh[b][nsl, :])
                posn = sb.tile([58, 128], f32, tag="posn")
                nc.sync.dma_start(out=posn[:], in_=ins[f"posn{b}"][:, nsl])
                pvt = psB.tile([H, 128], f32, tag="psB")
                nc.tensor.transpose(pvt[:], vold[:], ident[:])
                vT = sb.tile([H, 128], f32, tag="vT")
                nc.scalar.activation(vT[:], pvt[:], AF.Copy)
                ph = psB.tile([H, 128], f32, tag="psB")
                nc.tensor.matmul(ph[:], lhsT=wnv[k][:], rhs=vT[:], start=True, stop=False)
                nc.tensor.matmul(ph[:], lhsT=wnp[k][:], rhs=posn[:], start=False, stop=False)
                nc.tensor.matmul(ph[:], lhsT=wna[k][:], rhs=agg[:, nsl], start=False, stop=True)
                hid = sb.tile([H, 128], f32, tag="hidn")
                nc.scalar.activation(hid[:], ph[:], AF.Relu)
                pyn = psB.tile([128, H], f32, tag="psB")
                nc.tensor.matmul(pyn[:], lhsT=hid[:], rhs=w1n[k][:], start=True, stop=False)
                nc.tensor.matmul(pyn[:], lhsT=ones_row[:], rhs=b1n[k][:], start=False, stop=True)
                rstd, nmr = layernorm_cols([pyn[:]])
                normv = sb.tile([128, H], f32, tag="normv")
                nc.scalar.activation(normv[:], pyn[:], AF.Identity,
                                     bias=nmr[:, 0:1], scale=rstd[:, 0:1])
                t = sb.tile([128, H], f32, tag="tgam")
                nc.vector.tensor_tensor(t[:], in0=normv[:], in1=gamv[k][:], op=OP.mult)
                vnew = sb.tile([128, H], f32, tag="vnew")
                nc.vector.tensor_tensor(vnew[:], in0=vold[:], in1=t[:], op=OP.add)
                if k == NR - 1:
                    nc.sync.dma_start(out=outs[f"Vout{b}"][nsl, :], in_=vnew[:])
                else:
                    nc.sync.dma_start(out=vsh[b][nsl, :], in_=vnew[:])
            if k < NR - 1:
                nc.gpsimd.collective_compute(
                    "AllGather", OP.bypass, replica_groups=RG,
                    ins=[vsh[b].ap()], outs=[vtab[b].ap()])

    for p in (psB, psA, aggp, gath, sb, consts):
        p.release()


# ---------------------------------------------------------------------------
# runner
# ---------------------------------------------------------------------------

def build_module(cfg, debug=False):
    """Build + tile-schedule the Bass module. Returns nc."""
    import concourse.bacc as bacc
    import concourse.tile as tile
    from concourse import mybir

    B, NCORES, ESH, CHT, NSLOT, NR = (
        cfg["B"], cfg["NCORES"], cfg["ESH"], cfg["CHT"], cfg["NSLOT"], cfg["NROUNDS"],
    )
    f32 = mybir.dt.float32

    nc = bacc.Bacc("TRN2", target_bir_lowering=False, debug=debug,
                   num_devices=NCORES)

    ins = {}
    outs = {}

    def din(name, shape, dtype=f32):
        ins[name] = nc.dram_tensor(name, list(shape), dtype, kind="ExternalInput").ap()

    for b in range(B):
        din(f"soff{b}", (128, CHT), mybir.dt.int32)
        din(f"ohT{b}", (128, ESH))
        din(f"poss{b}", (57, ESH))
        din(f"posr{b}", (56, ESH))
        din(f"geom{b}", (4, ESH))
        din(f"oh{b}", (128, CHT * 128))
        din(f"nodef{b}", (13, NSLOT))
        din(f"posn{b}", (58, NSLOT))
    wshapes = {"wgn": (13, H), "w1gn": (H, H), "b1gn": (1, H),
               "wge": (4, H), "w1ge": (H, H), "b1ge": (1, H)}
    for k in range(NR):
        wshapes.update({
            f"wvs{k}": (H, H), f"wvr{k}": (H, H), f"wps{k}": (57, H),
            f"wpr{k}": (56, H), f"wee{k}": (H, H), f"w1e{k}": (H, H),
            f"b1e{k}": (1, H), f"game{k}": (H, 1),
            f"wnv{k}": (H, H), f"wnp{k}": (58, H), f"wna{k}": (H, H),
            f"w1n{k}": (H, H), f"b1n{k}": (1, H), f"gamv{k}": (H, H),
        })
    for name, shp in wshapes.items():
        din(name, shp)
    for b in range(B):
        outs[f"Vout{b}"] = nc.dram_tensor(
            f"Vout{b}", [NSLOT, H], f32, kind="ExternalOutput").ap()
        outs[f"Eout{b}"] = nc.dram_tensor(
            f"Eout{b}", [128, ESH], f32, kind="ExternalOutput").ap()

    with tile.TileContext(nc) as tc:
        build_trace(tc, ins, outs, cfg)
    nc.compile()
    return nc


def make_in_maps(per_core_b, W, cfg):
    in_maps = []
    for c in range(cfg["NCORES"]):
        m = {}
        for b in range(cfg["B"]):
            d = per_core_b[b][c]
            m[f"soff{b}"] = d["soff"].astype(np.int32)
            m[f"ohT{b}"] = d["ohT"]
            m[f"poss{b}"] = d["poss"]
            m[f"posr{b}"] = d["posr"]
            m[f"geom{b}"] = d["geom"]
            m[f"oh{b}"] = d["oh"]
            m[f"nodef{b}"] = d["nodef"]
            m[f"posn{b}"] = d["posn"]
        for name, arr in W.items():
            if not name.startswith("_"):
                m[name] = arr
        in_maps.append(m)
    return in_maps


def run_device(per_core_b, W, cfg, trace=False):
    """Compile + run on the 8 cores. Returns BassKernelResults."""
    from concourse.bass_utils import run_bass_kernel_spmd

    nc = build_module(cfg)
    in_maps = make_in_maps(per_core_b, W, cfg)
    res = run_bass_kernel_spmd(nc, in_maps, core_ids=list(range(cfg["NCORES"])),
                               trace=trace)
    return res


def kernel(mesh_pos, edges, states, node_type, pos_enc, params):
    mesh_pos = np.asarray(mesh_pos, dtype=np.float32)
    edges_np = np.asarray(edges)
    states = np.asarray(states, dtype=np.float32)
    node_type = np.asarray(node_type, dtype=np.float32)
    pos_enc = np.asarray(pos_enc, dtype=np.float32)

    B, N, _ = mesh_pos.shape
    Ne = edges_np.shape[1]

    wpc0 = int(math.ceil((N / 8) / 128))
    ew0 = int(math.ceil((Ne / 8) / wpc0 / 128)) * 128
    cfg = per_core_b = None
    for wpc, ew in [(wpc0, ew0), (wpc0, ew0 + 128), (wpc0 + 1, ew0 + 128)]:
        try:
            cfg = make_cfg(B, N, Ne, wpc=wpc, ew=ew)
            per_core_b = [
                prep_batch(b, edges_np, mesh_pos, states, node_type, pos_enc, cfg)
                for b in range(B)]
            break
        except AssertionError:
            cfg = per_core_b = None
    assert cfg is not None, "no feasible partition config"
    W = fold_weights(params)

    res = run_device(per_core_b, W, cfg, trace=False)
    results = res.results

    V = np.zeros((B, N, H), dtype=np.float32)
    E = np.zeros((B, Ne, H), dtype=np.float32)
    for c in range(cfg["NCORES"]):
        out = results[c]
        for b in range(B):
            d = per_core_b[b][c]
            V[b, d["node_ids"]] = out[f"Vout{b}"][d["node_slots"]] + W["_Bv"]
            E[b, d["edge_ids"]] = out[f"Eout{b}"][:, d["edge_slots"]].T + W["_Be"]
    kernel._last_res = res
    return V, E
